# revision 1
# baseline (speedup 1.0000x reference)
"""Masked dot-product attention (ESIM masked_softmax) Trainium2 Bass kernel.

Math (per batch):
    s   = q @ k^T ; t = s * m  (== q @ (k*m)^T, exact since m is 0/1)
    p   = exp(t) * m / sum_k(exp(t) * m)   (max-subtraction cancels; |s|<~50
                                            so exp() stays in fp32 range)
    out = p @ v = (exp(t) @ [v*m | m]) -> numerator | denominator

Device mapping (per core, 2 batches, data-parallel over 8 cores):
  - masked key rows are compacted away on the host (kept rows first, zero-mask
    padding to LKC=1792), shrinking every O(Lq*Lk) stage by ~12%.
  - scores are computed TRANSPOSED (k on partitions, q free) so exp(s^T) is
    directly the lhsT of the PV matmul; no O(Lq*Lk) transposes.
  - k*m / q are PE-transposed once per batch ([128,128] fp32 tiles), with q
    duplicated into both partition halves and k-blocks packed in pairs so the
    K=64 score matmuls row-tile two-at-a-time (~218ns per pair of N=512
    bf16 matmuls).
  - S matmul: 3 bf16 passes over hi/lo split operands (qh*kh + qh*kl + ql*kh)
    = within ~2^-16 of a full fp32 matmul at bf16 speed with LDWEIGHTS
    hidden. ATT_S_MODE=f32r selects a single fp22 pass instead.
  - PV uses float32r (fp22) with stationary [v*m | m]: column 64 of the
    accumulated output is the softmax denominator for free.
  - out^T [65, Lq] is PE-transposed back in 128-column chunks and normalized
    with a per-partition reciprocal multiply.
"""

import os
import sys

import numpy as np

sys.path.insert(0, "/opt/trn_rl_repo")

import concourse.bacc as bacc
import concourse.bass as bass
import concourse.mybir as mybir
import concourse.tile as tile
from concourse import bass_utils
from concourse.masks import make_identity

B, LQ, LK, D = 16, 2048, 2048, 64
NCORES = 8
PB = B // NCORES  # batches per core
P = 128
NQB = LQ // P  # 16 q-blocks

S_MODE = os.environ.get("ATT_S_MODE", "bf16_3p")  # "bf16_3p" | "f32r"
PV_MODE = os.environ.get("ATT_PV_MODE", "f32r")  # "f32r" | "fp32"
COMPACT = os.environ.get("ATT_COMPACT", "1") == "1"
LKC = 1792  # compacted key length (14 blocks); used when counts allow

F32 = mybir.dt.float32
F32R = mybir.dt.float32r
BF16 = mybir.dt.bfloat16
EXP = mybir.ActivationFunctionType.Exp


class _BatchCtx:
    pass


def _attention_core(tc, q_d, k_d, v_d, m_d, o_d, nkb):
    """Emit the per-core program. All dram handles are per-core shards."""
    nc = tc.nc
    npair = nkb // 2
    pools = []

    def pool(name, bufs, space="SBUF"):
        p = tc.alloc_tile_pool(name=name, bufs=bufs, space=space)
        pools.append(p)
        return p

    singles = pool("singles", 1)
    stage = pool("stage", 2)
    main = pool("main", 2)
    wtp = pool("wt", 12)
    outp = pool("outp", 2)
    smalls = pool("smalls", 4)

    ps_s = pool("ps_s", 3, space="PSUM")  # 3 x [128,1024] = 6 banks
    ps_pv = pool("ps_pv", 2, space="PSUM")  # 2 x 1-bank slots (pv chunks + nat)

    ident = singles.tile([P, P], F32, tag="ident")
    make_identity(nc, ident)

    three = S_MODE == "bf16_3p"
    sdt = F32 if three else F32R

    def prep_io(b, use_act_ring=False):
        bc = _BatchCtx()
        bc.m_sb = stage.tile([P, nkb], F32, tag="m", name=f"m_sb{b}")
        nc.sync.dma_start(out=bc.m_sb, in_=m_d[b].rearrange("(t p) -> p t", p=P))
        ksrc = k_d[b].rearrange("(t p) d -> p t d", p=P)
        bc.knat = stage.tile([P, nkb, D], F32, tag="knat", name=f"knat{b}")
        h0 = 2 * ((npair + 1) // 2)  # covers the k-pairs of transpose group 0
        nc.gpsimd.dma_start(out=bc.knat[:, :h0, :], in_=ksrc[:, :h0, :])
        nc.gpsimd.dma_start(out=bc.knat[:, h0:, :], in_=ksrc[:, h0:, :])
        bc.qdup = stage.tile([P, NQB, 2, D], F32, tag="qdup", name=f"qdup{b}")
        qsrc = q_d[b].rearrange("(t p) d -> p t d", p=P)
        if use_act_ring:
            # batch-0 head fast path: tiny dedicated loads for the first
            # k-pair and first 4 q-blocks so the first S matmul issues early.
            bc.kf = stage.tile([P, 2, D], F32, tag="kf", name=f"kf{b}")
            nc.sync.dma_start(out=bc.kf, in_=ksrc[:, 0:2, :])
            nc.sync.dma_start(out=bc.qdup[:, 0:4, 0, :], in_=qsrc[:, 0:4, :])
            nc.scalar.dma_start(out=bc.qdup[:, 0:4, 1, :], in_=qsrc[:, 0:4, :])
        for g in range(2):
            gs = slice(4 if (use_act_ring and g == 0) else g * 8, (g + 1) * 8)
            nc.sync.dma_start(out=bc.qdup[:, gs, 0, :], in_=qsrc[:, gs, :])
            # batch 0 head: second copy on the idle ACT HWDGE ring, parallel.
            # Later batches must not touch the ACT ring (it would stall exp).
            eng = nc.scalar if use_act_ring else nc.gpsimd
            eng.dma_start(out=bc.qdup[:, gs, 1, :], in_=qsrc[:, gs, :])
        bc.vnat = stage.tile([P, nkb, D], F32, tag="vnat", name=f"vnat{b}")
        nc.gpsimd.dma_start(out=bc.vnat, in_=v_d[b].rearrange("(t p) d -> p t d", p=P))
        return bc

    def prep_units(b, bc):
        """Closures emitting prep compute; callable in order, spreadable."""
        km = bc.knat  # host pre-multiplied: knat already is k*m
        bc.kmT = main.tile([P, npair, P], sdt, tag="kmT", name=f"kmT{b}")
        if three:
            bc.kmTh = main.tile([P, npair, P], BF16, tag="kmTh", name=f"kmTh{b}")
            bc.kmTl = main.tile([P, npair, P], BF16, tag="kmTl", name=f"kmTl{b}")
        bc.qT = main.tile([P, LQ], sdt, tag="qT", name=f"qT{b}")
        if three:
            bc.qTh = main.tile([P, LQ], BF16, tag="qTh", name=f"qTh{b}")
            bc.qTl = main.tile([P, LQ], BF16, tag="qTl", name=f"qTl{b}")
        bc.vme = stage.tile(
            [P, nkb, D + 1], F32R if PV_MODE == "f32r" else F32, tag="vme",
            name=f"vme{b}",
        )
        bc.out_sb = outp.tile([P, NQB, D], F32, tag="osb", name=f"osb{b}")

        h0 = 2 * ((npair + 1) // 2)
        fast = hasattr(bc, "kf")

        def u_fast():
            # first k-pair + first 4 q-blocks: unblocks S(j=0, c=0) early
            tr = ps_s.tile([P, 5 * P], F32, tag="s", name=f"trf{b}")
            nc.tensor.transpose(tr[:, 0:P], bc.kf, ident)
            for i in range(4):
                nc.tensor.transpose(
                    tr[:, (i + 1) * P : (i + 2) * P], bc.qdup[:, i], ident
                )
            dst = bc.kmT[:, 0:1, :].rearrange("p a b -> p (a b)")
            nc.scalar.copy(dst, tr[:, 0:P])
            nc.scalar.copy(bc.qT[:, 0:512], tr[:, P:])
            if three:
                dh = bc.kmTh[:, 0:1, :].rearrange("p a b -> p (a b)")
                nc.vector.tensor_copy(dh, dst)
                nc.vector.tensor_sub(
                    bc.kmTl[:, 0:1, :].rearrange("p a b -> p (a b)"), dst, dh
                )
                nc.vector.tensor_copy(bc.qTh[:, 0:512], bc.qT[:, 0:512])
                nc.vector.tensor_sub(
                    bc.qTl[:, 0:512], bc.qT[:, 0:512], bc.qTh[:, 0:512]
                )

        def u_kmT(grp, act_copy=False):
            jlo = grp * (npair + 1) // 2
            jhi = npair if grp else (npair + 1) // 2
            if fast and grp == 0:
                jlo = 1
            def go():
                nj = jhi - jlo
                tr = ps_s.tile([P, nj * P], F32, tag="s", name=f"trk{b}_{grp}")
                for j in range(jlo, jhi):
                    nc.tensor.transpose(
                        tr[:, (j - jlo) * P : (j - jlo + 1) * P],
                        km[:, 2 * j : 2 * j + 2, :], ident,
                    )
                dst = bc.kmT[:, jlo:jhi, :].rearrange("p a b -> p (a b)")
                (nc.scalar.copy if act_copy else nc.vector.tensor_copy)(dst, tr)
                if three:
                    dh = bc.kmTh[:, jlo:jhi, :].rearrange("p a b -> p (a b)")
                    nc.vector.tensor_copy(dh, dst)
                    nc.vector.tensor_sub(
                        bc.kmTl[:, jlo:jhi, :].rearrange("p a b -> p (a b)"),
                        dst, dh,
                    )
            return go

        def u_qT(g, ilo, ihi, act_copy=False):
            def go():
                tr = ps_s.tile(
                    [P, (ihi - ilo) * P], F32, tag="s", name=f"trq{b}_{g}_{ilo}"
                )
                for i in range(ilo, ihi):
                    t = g * 8 + i
                    nc.tensor.transpose(
                        tr[:, (i - ilo) * P : (i - ilo + 1) * P], bc.qdup[:, t], ident
                    )
                half = slice((g * 8 + ilo) * P, (g * 8 + ihi) * P)
                (nc.scalar.copy if act_copy else nc.vector.tensor_copy)(
                    bc.qT[:, half], tr
                )
                if three:
                    nc.vector.tensor_copy(bc.qTh[:, half], bc.qT[:, half])
                    nc.vector.tensor_sub(
                        bc.qTl[:, half], bc.qT[:, half], bc.qTh[:, half]
                    )
            return go

        def u_vme():
            nc.vector.tensor_copy(bc.vme[:, :, 0:D], bc.vnat)
            nc.vector.tensor_copy(bc.vme[:, :, D], bc.m_sb[:, :])

        units = [
            u_kmT(0, act_copy=fast), u_qT(0, 4 if fast else 0, 8, act_copy=fast),
            u_kmT(1), u_vme, u_qT(1, 0, 4), u_qT(1, 4, 8),
        ]
        if fast:
            units.insert(0, u_fast)
        else:
            units.insert(1, u_qT(0, 0, 4))
            units[2] = u_qT(0, 4, 8)
        return units

    def main_half(b, bc, h, side_work=(), finals_out=None):
        side = list(side_work)
        pvc = [
            ps_pv.tile([65, 512], F32, tag="pv", name=f"pv{b}_{h}_{c}")
            for c in range(2)
        ]
        if three:
            passes = [
                (bc.kmTh, bc.qTh, True, False),
                (bc.kmTl, bc.qTh, False, False),
                (bc.kmTh, bc.qTl, False, True),
            ]
        else:
            passes = [(bc.kmT, bc.qT, True, True)]
        wdt = F32R if PV_MODE == "f32r" else F32

        def emit_pv(j, wA, wB):
            # c innermost: consecutive matmuls alternate PSUM banks, so the
            # accumulate never waits on its own bank's drain.
            for kb, w in ((2 * j, wA), (2 * j + 1, wB)):
                for c in range(2):
                    cs = slice(c * 512, (c + 1) * 512)
                    nc.tensor.matmul(
                        pvc[c], bc.vme[:, kb, :], w[:, cs],
                        start=(kb == 0), stop=(kb == nkb - 1),
                    )

        pend = []
        for j in range(npair):
            sA = ps_s.tile([P, 1024], F32, tag="s", name=f"sA{b}_{h}_{j}")
            sB = ps_s.tile([P, 1024], F32, tag="s", name=f"sB{b}_{h}_{j}")
            # c innermost: consecutive same-side matmuls alternate banks (no
            # accumulate drain-wait) and A/B stay adjacent so they row-pair.
            for kt, qt, st, sp in passes:
                for c in range(2):
                    qs = slice(h * 1024 + c * 512, h * 1024 + (c + 1) * 512)
                    cs = slice(c * 512, (c + 1) * 512)
                    nc.tensor.matmul(
                        sA[:, cs], kt[0:64, j, :], qt[0:64, qs],
                        start=st, stop=sp, tile_position=(0, 0),
                    )
                    nc.tensor.matmul(
                        sB[:, cs], kt[64:128, j, :], qt[64:128, qs],
                        start=st, stop=sp, tile_position=(64, 0),
                    )
            wA = wtp.tile([P, 1024], wdt, tag="wt", name=f"wA{b}_{h}_{j}")
            wB = wtp.tile([P, 1024], wdt, tag="wt", name=f"wB{b}_{h}_{j}")
            nc.scalar.activation(out=wA, in_=sA, func=EXP)
            nc.scalar.activation(out=wB, in_=sB, func=EXP)
            # PV lags two j-groups: its exps finished long ago, so the
            # in-order PE never stalls on ScalarE here.
            pend.append((j, wA, wB))
            if len(pend) > 2:
                emit_pv(*pend.pop(0))
            if side:
                side.pop(0)()
        while pend:
            emit_pv(*pend.pop(0))
        while side:
            side.pop(0)()

        # drain: copy the accumulators out (freeing the pv slots for the next
        # half) and hand the transpose-back/normalize work to the caller so it
        # can interleave into the next half's stream instead of starving ACT.
        outT = outp.tile([D + 1, 1024], F32, tag="outT", name=f"outT{b}_{h}")
        for c in range(2):
            nc.vector.tensor_copy(outT[:, c * 512 : (c + 1) * 512], pvc[c])

        def fin(q0):
            def go():
                for qb in range(q0, q0 + 4):
                    nat = ps_s.tile([P, D + 1], F32, tag="s", name=f"nat{b}_{h}_{qb}")
                    nc.tensor.transpose(
                        nat, outT[:, qb * P : (qb + 1) * P],
                        ident[0 : D + 1, 0 : D + 1],
                    )
                    rc = smalls.tile([P, 1], F32, tag="rc", name=f"rc{b}_{h}_{qb}")
                    nc.vector.reciprocal(rc, nat[:, D : D + 1])
                    nc.vector.tensor_scalar_mul(
                        bc.out_sb[:, h * 8 + qb, :], nat[:, 0:D], rc
                    )
            return go

        if finals_out is None:
            fin(0)()
            fin(4)()
        else:
            finals_out.extend([fin(0), fin(4)])

    def store(b, bc):
        nc.sync.dma_start(
            out=o_d[b].rearrange("(t p) d -> p t d", p=P), in_=bc.out_sb
        )

    # Interleave batch 1's prep into batch 0's stream: no PE bubble at the
    # batch boundary, and prep transposes spread out so HAM stays warm.
    # Batch 1's input DMAs are issued after batch 0's so they don't delay
    # the head-critical loads on the shared rings. Only the units needed by
    # the first few S matmuls run before the main loop; the rest spread as
    # per-iteration side work so the PE reaches the first S matmul early.
    bcs = [prep_io(0, use_act_ring=True)]
    # u0: [fast, kmT0, qT0b, kmT1, vme, qT1a, qT1b]
    u0 = prep_units(0, bcs[0])
    for u in u0[:3]:
        u()
    if PB > 1:
        bcs.append(prep_io(1))
        # u1: [kmT0, qT0a, qT0b, kmT1, vme, qT1a, qT1b]
        u1 = prep_units(1, bcs[1])
    else:
        u1 = []
    side00 = [u0[4], u0[3], u0[5], u0[6]] + u1[:3]  # vme first (PV needs it)
    f = []
    main_half(0, bcs[0], 0, side_work=side00, finals_out=f)
    f2 = []
    main_half(0, bcs[0], 1, side_work=f + u1[3:], finals_out=f2)
    if PB > 1:
        f3 = []
        main_half(1, bcs[1], 0, side_work=f2, finals_out=f3)
        store(0, bcs[0])
        main_half(1, bcs[1], 1, side_work=f3, finals_out=None)
        store(1, bcs[1])
    else:
        for u in f2:
            u()
        store(0, bcs[0])

    for p in reversed(pools):
        p.release()


_NC_CACHE = {}


def _build_nc(nkb):
    if nkb in _NC_CACHE:
        return _NC_CACHE[nkb]
    lk = nkb * P
    nc = bacc.Bacc(None, target_bir_lowering=False, debug=False)
    q_d = nc.dram_tensor("q", [PB, LQ, D], F32, kind="ExternalInput")
    k_d = nc.dram_tensor("k", [PB, lk, D], F32, kind="ExternalInput")
    v_d = nc.dram_tensor("v", [PB, lk, D], F32, kind="ExternalInput")
    m_d = nc.dram_tensor("m", [PB, lk], F32, kind="ExternalInput")
    o_d = nc.dram_tensor("out", [PB, LQ, D], F32, kind="ExternalOutput")
    with tile.TileContext(nc) as tc:
        _attention_core(tc, q_d, k_d, v_d, m_d, o_d, nkb)
    nc.compile()
    _NC_CACHE[nkb] = nc
    return nc


def kernel(q, k, v, v_mask, _trace=False, _tmpdir=None):
    q = np.ascontiguousarray(q, dtype=np.float32)
    k = np.ascontiguousarray(k, dtype=np.float32)
    v = np.ascontiguousarray(v, dtype=np.float32)
    v_mask = np.ascontiguousarray(v_mask, dtype=np.float32)
    assert q.shape == (B, LQ, D), q.shape

    # fold the 0/1 mask into k and v on the host (exact; removes the device
    # DVE mask-multiply chain from the critical path)
    k = k * v_mask[:, :, None]
    v = v * v_mask[:, :, None]
    counts = (v_mask > 0.5).sum(axis=1)
    if COMPACT and counts.max() <= LKC:
        # kept key rows first (stable), zero-mask padding after; the packed
        # mask makes padded rows contribute exactly 0 on device.
        order = np.argsort(v_mask <= 0.5, axis=1, kind="stable")[:, :LKC]
        kk = np.take_along_axis(k, order[:, :, None], axis=1)
        vv = np.take_along_axis(v, order[:, :, None], axis=1)
        mm = np.take_along_axis(v_mask, order, axis=1)
        nkb = LKC // P
    else:
        kk, vv, mm = k, v, v_mask
        nkb = LK // P

    nc = _build_nc(nkb)
    in_maps = [
        {
            "q": np.ascontiguousarray(q[i * PB : (i + 1) * PB]),
            "k": np.ascontiguousarray(kk[i * PB : (i + 1) * PB]),
            "v": np.ascontiguousarray(vv[i * PB : (i + 1) * PB]),
            "m": np.ascontiguousarray(mm[i * PB : (i + 1) * PB]),
        }
        for i in range(NCORES)
    ]
    res = bass_utils.run_bass_kernel_spmd(
        nc, in_maps, core_ids=list(range(NCORES)), trace=_trace, tmpdir=_tmpdir
    )
    out = np.concatenate([r["out"] for r in res.results], axis=0)
    if _trace:
        kernel.last_results = res
    return out

